# revision 14
# baseline (speedup 1.0000x reference)
"""Trainium2 Bass kernel for nn_ChittaEncoder (retrieval kNN encoder).

Reference computation (per row of x):
    q      = x @ W.T                         [B, 128]
    scores = (q @ seeds.T) / sqrt(128)       [B, 500]
    top4, idx4 = top_k(scores, 4)
    attn   = softmax(top4)                   [B, 4]
    field  = sum_k attn[k] * seeds[idx4[k]]  [B, 128]
    returns (field, attn)

Strategy (pure data-parallel over 8 NeuronCores, batch-sharded):
  - Host folds the two matmuls into one:  scores = x @ C.T  with
    C = seeds @ W / sqrt(128)  (500x128), and pre-transposes/splits x into
    bf16 hi/lo halves so the device does a 3-term bf16 matmul (hi*hi +
    hi*lo + lo*hi) that is fp32-accurate.
  - Device per 128-row tile: PE matmul -> scores PSUM [128,500];
    DVE max8/max_index read PSUM directly and find top-4 values+indices;
    softmax on ACT (exp with per-partition bias + accumulated Z) and DVE
    (reciprocal); the 4 selected seed rows are fetched with two GPSIMD
    indirect DMAs from a host-built pair table T2[a*500+b]=[seeds[a]|seeds[b]]
    (HW indirect DMA supports exactly one offset + one contiguous run per
    partition); DVE does the attn-weighted sum in bf16.
"""

import math
import os
import sys

import numpy as np

for _p in ("/opt/trn_rl_repo", "/opt/pypackages"):
    if _p not in sys.path and os.path.isdir(_p):
        sys.path.append(_p)

import ml_dtypes  # noqa: E402

import concourse.bass as bass  # noqa: E402
import concourse.mybir as mybir  # noqa: E402
from concourse import bacc  # noqa: E402
from concourse import bass_utils  # noqa: E402
from concourse.tile import TileContext  # noqa: E402

BF16 = ml_dtypes.bfloat16

D_MODEL = 128
N_SEEDS = 500
TOP_K = 4
BATCH = 524288
N_CORES = 8
B_CORE = BATCH // N_CORES  # 65536

P = 128  # partitions / tile rows
GRP = 8  # tiles per group (batched stores)


def build_nc(b_core: int = B_CORE, grp: int = GRP):
    """Build the Bass program for one core processing b_core rows."""
    f32 = mybir.dt.float32
    bf16 = mybir.dt.bfloat16
    u32 = mybir.dt.uint32

    n_tiles = b_core // P
    assert b_core % P == 0 and n_tiles % grp == 0
    n_grps = n_tiles // grp

    nc = bacc.Bacc("TRN2", target_bir_lowering=False,
                   dynamic_dma_scratch_size=1 << 16)

    xt_hi = nc.dram_tensor("xt_hi", [P, b_core], bf16, kind="ExternalInput")
    xt_lo = nc.dram_tensor("xt_lo", [P, b_core], bf16, kind="ExternalInput")
    ct_hi = nc.dram_tensor("ct_hi", [P, N_SEEDS], bf16, kind="ExternalInput")
    ct_lo = nc.dram_tensor("ct_lo", [P, N_SEEDS], bf16, kind="ExternalInput")
    pair_tab = nc.dram_tensor(
        "pair_tab", [N_SEEDS * N_SEEDS, 2 * D_MODEL], bf16, kind="ExternalInput"
    )

    field_out = nc.dram_tensor("field", [b_core, D_MODEL], f32, kind="ExternalOutput")
    attn_out = nc.dram_tensor("attn", [b_core, TOP_K], f32, kind="ExternalOutput")

    # DRAM views for grouped IO: row index = s*GRP*P + t*P + p
    field_v = field_out[:].rearrange("(s t p) d -> s p t d", p=P, t=grp)
    attn_v = attn_out[:].rearrange("(s t p) k -> s p t k", p=P, t=grp)
    xt_hi_v = xt_hi[:].rearrange("p (s t b) -> s p t b", t=grp, b=P)
    xt_lo_v = xt_lo[:].rearrange("p (s t b) -> s p t b", t=grp, b=P)

    with TileContext(nc) as tc:
        with (
            tc.tile_pool(name="const", bufs=1) as const_pool,
            tc.tile_pool(name="xt", bufs=3) as xt_pool,
            tc.tile_pool(name="psum", bufs=8, space="PSUM") as psum_pool,
            tc.tile_pool(name="sel", bufs=8) as sel_pool,
            tc.tile_pool(name="grp", bufs=2) as grp_pool,
            tc.tile_pool(name="gat", bufs=6) as gat_pool,
            tc.tile_pool(name="wsum", bufs=4) as wsum_pool,
        ):
            ct_hi_sb = const_pool.tile([P, N_SEEDS], bf16, tag="ct_hi")
            ct_lo_sb = const_pool.tile([P, N_SEEDS], bf16, tag="ct_lo")
            nc.sync.dma_start(ct_hi_sb[:], ct_hi[:])
            nc.sync.dma_start(ct_lo_sb[:], ct_lo[:])

            for s in range(n_grps):
                xh = xt_pool.tile([P, grp, P], bf16, tag="xh")
                xl = xt_pool.tile([P, grp, P], bf16, tag="xl")
                nc.sync.dma_start(xh[:], xt_hi_v[s])
                nc.sync.dma_start(xl[:], xt_lo_v[s])

                e4_grp = grp_pool.tile([P, grp, TOP_K], f32, tag="e4")
                attn_grp = grp_pool.tile([P, grp, TOP_K], f32, tag="attn")
                field_grp = grp_pool.tile([P, grp, D_MODEL], f32, tag="field")

                for t in range(grp):
                    ps = psum_pool.tile([P, N_SEEDS], f32, tag="ps")
                    nc.tensor.matmul(ps[:], xh[:, t], ct_hi_sb[:], start=True, stop=False)
                    nc.tensor.matmul(ps[:], xh[:, t], ct_lo_sb[:], start=False, stop=False)
                    nc.tensor.matmul(ps[:], xl[:, t], ct_hi_sb[:], start=False, stop=True)

                    m8 = sel_pool.tile([P, 8], f32, tag="m8")
                    i8 = sel_pool.tile([P, 8], u32, tag="i8")
                    nc.vector.max(out=m8[:], in_=ps[:])
                    nc.vector.max_index(out=i8[:], in_max=m8[:], in_values=ps[:])

                    negm1 = sel_pool.tile([P, 1], f32, tag="negm1")
                    nc.scalar.activation(
                        negm1[:], m8[:, :1], mybir.ActivationFunctionType.Copy,
                        scale=-1.0,
                    )
                    z = sel_pool.tile([P, 1], f32, tag="z")
                    nc.scalar.activation(
                        e4_grp[:, t], m8[:, :TOP_K],
                        mybir.ActivationFunctionType.Exp,
                        bias=negm1[:], accum_out=z[:],
                    )
                    rz = sel_pool.tile([P, 1], f32, tag="rz")
                    nc.vector.reciprocal(rz[:], z[:])
                    nc.scalar.activation(
                        attn_grp[:, t], e4_grp[:, t],
                        mybir.ActivationFunctionType.Copy, scale=rz[:],
                    )

                    # Pair-table gather: offsets o = a*500 + b fetch the
                    # concatenated [seeds[a] | seeds[b]] row (one offset, one
                    # contiguous 256-elem run per partition -- the only
                    # indirect-DMA shape the HW DGE supports).
                    pidx = sel_pool.tile([P, 2], u32, tag="pidx")
                    for h in range(2):
                        nc.vector.scalar_tensor_tensor(
                            out=pidx[:, h : h + 1],
                            in0=i8[:, 2 * h : 2 * h + 1],
                            scalar=float(N_SEEDS),
                            in1=i8[:, 2 * h + 1 : 2 * h + 2],
                            op0=mybir.AluOpType.mult,
                            op1=mybir.AluOpType.add,
                        )
                    gat = gat_pool.tile([P, 2, 2 * D_MODEL], bf16, tag="gat")
                    for h in range(2):
                        nc.gpsimd.indirect_dma_start(
                            out=gat[:, h],
                            out_offset=None,
                            in_=pair_tab[:],
                            in_offset=bass.IndirectOffsetOnAxis(
                                ap=pidx[:, h : h + 1], axis=0
                            ),
                            bounds_check=N_SEEDS * N_SEEDS - 1,
                            oob_is_err=False,
                        )

                    acc = wsum_pool.tile([P, D_MODEL], bf16, tag="acc")
                    nc.vector.tensor_scalar_mul(
                        acc[:], gat[:, 0, :D_MODEL], attn_grp[:, t, 0:1]
                    )
                    for k in range(1, TOP_K - 1):
                        h, j = divmod(k, 2)
                        nc.vector.scalar_tensor_tensor(
                            out=acc[:],
                            in0=gat[:, h, j * D_MODEL : (j + 1) * D_MODEL],
                            scalar=attn_grp[:, t, k : k + 1],
                            in1=acc[:],
                            op0=mybir.AluOpType.mult,
                            op1=mybir.AluOpType.add,
                        )
                    nc.vector.scalar_tensor_tensor(
                        out=field_grp[:, t],
                        in0=gat[:, 1, D_MODEL:],
                        scalar=attn_grp[:, t, TOP_K - 1 : TOP_K],
                        in1=acc[:],
                        op0=mybir.AluOpType.mult,
                        op1=mybir.AluOpType.add,
                    )

                nc.sync.dma_start(field_v[s], field_grp[:])
                nc.sync.dma_start(attn_v[s], attn_grp[:])

    nc.compile()
    return nc


def host_prep(x: np.ndarray, W: np.ndarray, seeds: np.ndarray):
    """Host-side input marshaling: fold W into the seed table, transpose and
    bf16-split x per core."""
    x = np.asarray(x, dtype=np.float32)
    W = np.asarray(W, dtype=np.float32)
    seeds = np.asarray(seeds, dtype=np.float32)

    C = (seeds.astype(np.float64) @ W.astype(np.float64) / math.sqrt(D_MODEL)).astype(
        np.float32
    )  # [500, 128]
    CT = np.ascontiguousarray(C.T)  # [128, 500]
    ct_hi = CT.astype(BF16)
    ct_lo = (CT - ct_hi.astype(np.float32)).astype(BF16)
    seeds_bf = seeds.astype(BF16)
    pair_tab = np.empty((N_SEEDS, N_SEEDS, 2, D_MODEL), dtype=BF16)
    pair_tab[:, :, 0, :] = seeds_bf[:, None, :]
    pair_tab[:, :, 1, :] = seeds_bf[None, :, :]
    pair_tab = pair_tab.reshape(N_SEEDS * N_SEEDS, 2 * D_MODEL)

    in_maps = []
    for c in range(N_CORES):
        xs = x[c * B_CORE : (c + 1) * B_CORE]
        xT = np.ascontiguousarray(xs.T)  # [128, B_CORE]
        xt_hi = xT.astype(BF16)
        xt_lo = (xT - xt_hi.astype(np.float32)).astype(BF16)
        in_maps.append(
            {
                "xt_hi": xt_hi,
                "xt_lo": xt_lo,
                "ct_hi": ct_hi,
                "ct_lo": ct_lo,
                "pair_tab": pair_tab,
            }
        )
    return in_maps


_NC_CACHE: dict = {}


def get_nc(b_core: int = B_CORE, grp: int = GRP):
    key = (b_core, grp)
    if key not in _NC_CACHE:
        _NC_CACHE[key] = build_nc(b_core, grp)
    return _NC_CACHE[key]


def run_device(in_maps, trace: bool = False, **kwargs):
    nc = get_nc()
    return bass_utils.run_bass_kernel_spmd(
        nc, in_maps, core_ids=list(range(N_CORES)), trace=trace, **kwargs
    )


def kernel(x: np.ndarray, W: np.ndarray, seeds: np.ndarray):
    in_maps = host_prep(x, W, seeds)
    res = run_device(in_maps)
    field = np.concatenate([r["field"] for r in res.results], axis=0)
    attn = np.concatenate([r["attn"] for r in res.results], axis=0)
    return field.astype(np.float32), attn.astype(np.float32)
